# revision 34
# baseline (speedup 1.0000x reference)
"""Trainium2 Bass kernel for nn_ExpandOperator (banded scatter of a linear projection).

Reference semantics:
    pred = x @ W.T + b                      # (B, S, 2048)
    pred = pred.reshape(B, S, 64, 32)
    out[b, t, (t+s) % S, d] = pred[b, t, s, d]   # rest of out is zeros
    out shape: (B, S, S, 32) fp32  == 1 GiB

Sharding: 8 cores = (batch b in {0,1}) x (512-row seq chunk cc in {0..3}).

Key structure: the 1 GiB output is 96.9% structural zeros — only the
(B, S, 2048)-float band carries data, and every band value is just
pred[b, t, :].  So the device computes ONLY the dense projection
pred = x @ W.T for its 512 rows (bf16 in/out; tolerance is 2e-2, bf16
error here is ~1e-3) and returns it as a compact (512, 2048) tile.
The host unshards by scattering the band into an np.zeros output —
row t's band occupies flat columns [32*t, 32*t+2048) mod 65536 of
out[b, t], which for the 1985 non-wrapping rows is a single strided
(diagonal) view assignment; the 63 wrapping rows are split copies.
The bias add (exact fp32) also folds into the host scatter:
out band row = pred_row + b.

Device per core: load [W.T | x.T] packed bf16 (6 k-tiles of 128 rows,
one DMA each so matmuls start after the first ~1.8us), 96 bf16 matmuls
(128x128x512, k-outer over 8 concurrent PSUM banks so accumulation
overlaps the remaining loads), DVE PSUM->SBUF copies (fp32->bf16), and
4 per-row-block band stores.  ~6 MB of HBM traffic and ~20.5us of PE
time per core, vs 134 MB of DMA in the write-the-zeros formulation.

The walrus build only leaves room for ONE sync-wait per compute
instruction; _split_multi_waits() hoists extra waits into same-queue
NOPs (same-queue waits execute in order, so this is semantics-neutral).
"""

import numpy as np

import bass_rust
import concourse.bass as bass
import concourse.mybir as mybir
import concourse.tile as tile
from concourse.bass_utils import run_bass_kernel_spmd

F32 = mybir.dt.float32
BF16 = mybir.dt.bfloat16
NP_BF16 = mybir.dt.np(mybir.dt.bfloat16)


def _split_multi_waits(nc):
    """Walrus in this toolchain only leaves ONE sync-wait slot per
    instruction.  Tile's tail drain waits on every semaphore lane it used,
    which fails codegen.  Hoist all-but-one wait of any multi-wait
    instruction into single-wait NOPs on the same engine queue immediately
    before it - semantically identical (same-queue waits execute in order).
    """
    eng_by_type = {
        mybir.EngineType.SP: nc.sync,
        mybir.EngineType.PE: nc.tensor,
        mybir.EngineType.Activation: nc.scalar,
        mybir.EngineType.Pool: nc.gpsimd,
        mybir.EngineType.DVE: nc.vector,
    }
    tail_bb = nc.cur_bb.bb
    for f in nc.m.functions:
        for bb in f.blocks:
            il = bb.instructions
            i = 0
            while i < len(il):
                ins = il[i]
                si = getattr(ins, "sync_info", None)
                if si is not None and len(si.on_wait) > 1:
                    waits = list(si.on_wait)
                    for w in waits[:-1]:
                        nop = eng_by_type[ins.engine].nop(nofuse=True).ins
                        tail_bb.instructions.remove(nop)
                        nop.sync_info = bass_rust.SyncInfo(
                            on_wait=[w], on_update=[])
                        il.insert(i, nop)
                        i += 1
                    ins.sync_info = bass_rust.SyncInfo(
                        on_wait=[waits[-1]], on_update=list(si.on_update))
                i += 1


# Problem shapes (hardcoded per contract).
B = 2
S = 2048
D_IN = 768
MAX_SPAN = 64
SPAN_DIM = 32
N_OUT = MAX_SPAN * SPAN_DIM  # 2048
N_CORES = 8
CHUNKS = 4                   # seq chunks per batch (B * CHUNKS == N_CORES)
ROWS = S // CHUNKS           # 512 rows per core


def build_nc(rows=ROWS, d_in=D_IN, n_out=N_OUT, repeats=1, nw=512,
             split_loads=True, fine_stores=True, dual_copy=False,
             warm_pe=True, pe_only=False, diag=None, paired_psum=True,
             load_plan="per_k"):
    """Single-core Bass program (shared by all 8 cores via SPMD).

    Inputs (per core):
      wx : (d_in, n_out + rows) bf16, packed [W.T | x_chunk.T].
    Output:
      out: (rows, n_out) bf16 = pred = x_chunk @ W.T (no bias; host adds it).
    """
    kt = d_in // 128             # 6 contraction tiles
    mblk = rows // 128           # 4 row blocks
    nchunk = n_out // nw
    wcols = n_out + rows         # 2560
    half_mb = mblk // 2          # row blocks per PSUM generation
    # paired_psum: accumulate two 512-col chunks into one 2-bank PSUM
    # tile, halving DVE copy instructions and next-generation PE WAR
    # semaphore waits (16 -> 8 per repeat).
    tw = 2 * nw if paired_psum else nw   # PSUM tile width
    cpt = tw // nw                       # matmul chunks per tile
    ncp = nchunk // cpt                  # copies per row block

    nc = bass.Bass()
    wx = nc.dram_tensor("wx", [d_in, wcols], BF16, kind="ExternalInput")
    out = nc.dram_tensor("out", [rows, n_out], BF16, kind="ExternalOutput")

    wx_r = wx.rearrange("(k p) m -> p k m", p=128)    # (128, kt, wcols)
    out_r = out.rearrange("(mb p) c -> p mb c", p=128)  # (128, mblk, n_out)

    with tile.TileContext(nc) as tc:
        with (
            tc.tile_pool(name="wxp", bufs=2) as wxpool,
            tc.tile_pool(name="pred", bufs=2) as ppool,
            tc.tile_pool(name="psum", bufs=8 * 512 // tw,
                         space="PSUM") as pspool,
        ):
            if warm_pe:
                # The PE HAM clock-gate runs cold (1.2 GHz) until ~3.4us of
                # sustained activity.  Burn that window on dummy matmuls
                # over a zeroed tile (no load dependency, so they start at
                # t~0) so the real matmuls run at 2.4 GHz from the start.
                # Outside the repeat loop: steady-state cost is zero.
                wz = wxpool.tile([128, nw], BF16, name="wz", bufs=1)
                nc.vector.memset(wz[:], 0.0)
                wps = pspool.tile([128, tw], F32, name="ps")
                for _ in range(max(1, round(4200 / (nw + 6)))):
                    nc.tensor.matmul(wps[:, 0:nw], wz[:, 0:128], wz[:],
                                     start=True, stop=True)

            # Diagnostic builds: diag='mm' (96 matmuls only), 'ld_mm'
            # (loads + matmuls), 'mm_cp' (matmuls + copies + stores),
            # 'ld2_mm' (2 merged load DMAs + matmuls: tests per-DMA-op
            # cost), 'ldh_mm' (half the load bytes + matmuls: tests
            # byte-proportional cost).
            if pe_only:
                diag = "mm"
            do_loads = diag in (None, "ld_mm", "ld2_mm", "ldh_mm")
            do_drain = diag in (None, "mm_cp")
            if not do_loads:
                wx_pe = wxpool.tile([128, kt, wcols], BF16, name="wx_pe",
                                    bufs=1)
                nc.vector.memset(wx_pe[:], 0.0)

            for _rep in range(repeats):
                # Per-k-tile loads so the first matmul sweep can start
                # after ~1/6 of the load, overlapping the rest.  Split
                # across two DMA rings (scalar HWDGE + gpsimd SWDGE) so
                # k-tiles land ~2x faster during the fill; stores own the
                # sync ring exclusively, so cross-repeat load prefetch is
                # never FIFO-blocked behind a store.
                if not do_loads:
                    wx_sb = wx_pe
                else:
                    wx_sb = wxpool.tile([128, kt, wcols], BF16)
                    if diag == "ld2_mm":
                        h = kt // 2
                        nc.scalar.dma_start(wx_sb[:, 0:h, :],
                                            wx_r[:, 0:h, :])
                        nc.gpsimd.dma_start(wx_sb[:, h:kt, :],
                                            wx_r[:, h:kt, :])
                    elif load_plan == "scal3":
                        # 3 pair-tile DMAs, all on the scalar HWDGE ring:
                        # zero SWDGE ops (SWDGE fixed cost ~2us/op is the
                        # main measured per-op interference).  Pairs land
                        # at 3.6/7.2/10.7us -- just ahead of the k-sweep
                        # consumption schedule.
                        for j in range(0, kt, 2):
                            nc.scalar.dma_start(wx_sb[:, j:j + 2, :],
                                                wx_r[:, j:j + 2, :])
                    elif load_plan == "hwdge3":
                        # Loads on the two HWDGE rings (scalar + sync),
                        # stores displaced to gpsimd SWDGE.
                        nc.scalar.dma_start(wx_sb[:, 0:2, :],
                                            wx_r[:, 0:2, :])
                        nc.sync.dma_start(wx_sb[:, 2:4, :],
                                          wx_r[:, 2:4, :])
                        nc.scalar.dma_start(wx_sb[:, 4:kt, :],
                                            wx_r[:, 4:kt, :])
                    else:
                        lkt = kt // 2 if diag == "ldh_mm" else kt
                        for k in range(lkt):
                            eng = (nc.scalar
                                   if (k % 2 == 0 or not split_loads)
                                   else nc.gpsimd)
                            eng.dma_start(wx_sb[:, k, :], wx_r[:, k, :])

                pred = ppool.tile([128, mblk, n_out], BF16)
                st_eng = (nc.gpsimd if load_plan == "hwdge3"
                          else nc.sync)
                # Two generations of 8 concurrent PSUM banks; k-outer so
                # accumulation for all 8 chunks proceeds as k-tiles land.
                for half in range(2):
                    pss = [pspool.tile([128, tw], F32, name="ps")
                           for _ in range(half_mb * ncp)]
                    for k in range(kt):
                        for mi in range(half_mb):
                            mb = half * half_mb + mi
                            cs = n_out + mb * 128
                            for n in range(nchunk):
                                ps = pss[mi * ncp + n // cpt]
                                c0 = (n % cpt) * nw
                                nc.tensor.matmul(
                                    ps[:, c0:c0 + nw],
                                    wx_sb[:, k, cs:cs + 128],
                                    wx_sb[:, k, n * nw:(n + 1) * nw],
                                    start=(k == 0),
                                    stop=(k == kt - 1),
                                )
                    for mi in range(half_mb):
                        if not do_drain:
                            break
                        mb = half * half_mb + mi
                        for p in range(ncp):
                            # dual_copy alternates copies onto the scalar
                            # engine, but that queue also issues the even
                            # k-tile loads, so copies would FIFO-block the
                            # next repeat's load prefetch: off by default.
                            ps = pss[mi * ncp + p]
                            dst = pred[:, mb, p * tw:(p + 1) * tw]
                            if dual_copy and p % 2 == 1:
                                nc.scalar.copy(dst, ps[:])
                            else:
                                nc.vector.tensor_copy(dst, ps[:])
                            if fine_stores and ((p + 1) * tw) % 1024 == 0:
                                # Store each 1024-col slice of the row
                                # block as soon as it is copied (earlier
                                # drain, shorter tail than per-mb stores).
                                cs = (p + 1) * tw - 1024
                                st_eng.dma_start(
                                    out_r[:, mb, cs:cs + 1024],
                                    pred[:, mb, cs:cs + 1024])
                        if not fine_stores:
                            # Band store for this 128-row block.
                            st_eng.dma_start(out_r[:, mb, :],
                                             pred[:, mb, :])

    _split_multi_waits(nc)
    return nc


_CACHE = {}


def _get_nc():
    if "nc" not in _CACHE:
        _CACHE["nc"] = build_nc()
    return _CACHE["nc"]


def make_in_maps(x, W, b):
    """Host-side sharding: per-core packed [W.T | x_chunk.T] bf16."""
    x = np.asarray(x)
    W = np.asarray(W)
    WT = np.ascontiguousarray(W.T).astype(NP_BF16)    # (768, 2048)
    in_maps = []
    for c in range(N_CORES):
        bi, cc = divmod(c, CHUNKS)
        xs = x[bi, cc * ROWS:(cc + 1) * ROWS, :]
        wxc = np.empty((D_IN, N_OUT + ROWS), NP_BF16)
        wxc[:, :N_OUT] = WT
        wxc[:, N_OUT:] = np.ascontiguousarray(xs.T).astype(NP_BF16)
        in_maps.append({"wx": wxc})
    return in_maps


def unshard(results, b):
    """Scatter each core's dense band into the zero-filled full output.

    Row t's band occupies flat columns [32*t, 32*t+2048) mod 65536 of
    out[bi, t]; rows 0..1984 never wrap, so they're one strided
    (diagonal-view) assignment per batch.  Bias is added here in fp32.
    """
    b = np.asarray(b, dtype=np.float32)
    row_f = S * SPAN_DIM                       # 65536 floats per row
    nowrap = (row_f - N_OUT) // SPAN_DIM + 1   # 1985 non-wrapping rows
    out = np.zeros((B, S, S, SPAN_DIM), np.float32)
    for bi in range(B):
        band = np.concatenate(
            [np.asarray(results[bi * CHUNKS + cc]["out"]).astype(np.float32)
             for cc in range(CHUNKS)], axis=0)          # (2048, 2048)
        band += b[None, :]
        flat = out[bi].reshape(S * row_f)
        dv = np.lib.stride_tricks.as_strided(
            flat, shape=(nowrap, N_OUT),
            strides=((row_f + SPAN_DIM) * 4, 4))
        dv[:] = band[:nowrap]
        for t in range(nowrap, S):
            c0 = SPAN_DIM * t
            n1 = row_f - c0
            row = flat[t * row_f:(t + 1) * row_f]
            row[c0:] = band[t, :n1]
            row[:N_OUT - n1] = band[t, n1:]
    return out


def _run(nc, in_maps):
    return run_bass_kernel_spmd(nc, in_maps, list(range(N_CORES))).results


def kernel(x, W, b):
    x = np.asarray(x)
    W = np.asarray(W)
    b = np.asarray(b)
    nc = _get_nc()
    res = _run(nc, make_in_maps(x, W, b))
    return unshard(res, b)


# revision 36
# speedup vs baseline: 1.0079x; 1.0079x over previous
"""Trainium2 Bass kernel for nn_ExpandOperator (banded scatter of a linear projection).

Reference semantics:
    pred = x @ W.T + b                      # (B, S, 2048)
    pred = pred.reshape(B, S, 64, 32)
    out[b, t, (t+s) % S, d] = pred[b, t, s, d]   # rest of out is zeros
    out shape: (B, S, S, 32) fp32  == 1 GiB

Sharding: 8 cores = (batch b in {0,1}) x (512-row seq chunk cc in {0..3}).

Key structure: the 1 GiB output is 96.9% structural zeros — only the
(B, S, 2048)-float band carries data, and every band value is just
pred[b, t, :].  So the device computes ONLY the dense projection
pred = x @ W.T for its 512 rows (bf16 in/out; tolerance is 2e-2, bf16
error here is ~1e-3) and returns it as a compact (512, 2048) tile.
The host unshards by scattering the band into an np.zeros output —
row t's band occupies flat columns [32*t, 32*t+2048) mod 65536 of
out[b, t], which for the 1985 non-wrapping rows is a single strided
(diagonal) view assignment; the 63 wrapping rows are split copies.
The bias add (exact fp32) also folds into the host scatter:
out band row = pred_row + b.

Device per core: load [W.T | x.T] packed bf16 (6 k-tiles of 128 rows,
one DMA each so matmuls start after the first ~1.8us), 96 bf16 matmuls
(128x128x512, k-outer over 8 concurrent PSUM banks so accumulation
overlaps the remaining loads), DVE PSUM->SBUF copies (fp32->bf16), and
4 per-row-block band stores.  ~6 MB of HBM traffic and ~20.5us of PE
time per core, vs 134 MB of DMA in the write-the-zeros formulation.

The walrus build only leaves room for ONE sync-wait per compute
instruction; _split_multi_waits() hoists extra waits into same-queue
NOPs (same-queue waits execute in order, so this is semantics-neutral).
"""

import numpy as np

import bass_rust
import concourse.bass as bass
import concourse.mybir as mybir
import concourse.tile as tile
from concourse.bass_utils import run_bass_kernel_spmd

F32 = mybir.dt.float32
BF16 = mybir.dt.bfloat16
NP_BF16 = mybir.dt.np(mybir.dt.bfloat16)


def _split_multi_waits(nc):
    """Walrus in this toolchain only leaves ONE sync-wait slot per
    instruction.  Tile's tail drain waits on every semaphore lane it used,
    which fails codegen.  Hoist all-but-one wait of any multi-wait
    instruction into single-wait NOPs on the same engine queue immediately
    before it - semantically identical (same-queue waits execute in order).
    """
    eng_by_type = {
        mybir.EngineType.SP: nc.sync,
        mybir.EngineType.PE: nc.tensor,
        mybir.EngineType.Activation: nc.scalar,
        mybir.EngineType.Pool: nc.gpsimd,
        mybir.EngineType.DVE: nc.vector,
    }
    tail_bb = nc.cur_bb.bb
    for f in nc.m.functions:
        for bb in f.blocks:
            il = bb.instructions
            i = 0
            while i < len(il):
                ins = il[i]
                si = getattr(ins, "sync_info", None)
                if si is not None and len(si.on_wait) > 1:
                    waits = list(si.on_wait)
                    for w in waits[:-1]:
                        nop = eng_by_type[ins.engine].nop(nofuse=True).ins
                        tail_bb.instructions.remove(nop)
                        nop.sync_info = bass_rust.SyncInfo(
                            on_wait=[w], on_update=[])
                        il.insert(i, nop)
                        i += 1
                    ins.sync_info = bass_rust.SyncInfo(
                        on_wait=[waits[-1]], on_update=list(si.on_update))
                i += 1


# Problem shapes (hardcoded per contract).
B = 2
S = 2048
D_IN = 768
MAX_SPAN = 64
SPAN_DIM = 32
N_OUT = MAX_SPAN * SPAN_DIM  # 2048
N_CORES = 8
CHUNKS = 4                   # seq chunks per batch (B * CHUNKS == N_CORES)
ROWS = S // CHUNKS           # 512 rows per core


def build_nc(rows=ROWS, d_in=D_IN, n_out=N_OUT, repeats=1, nw=512,
             split_loads=True, fine_stores=True, dual_copy=False,
             warm_pe=True, pe_only=False, diag=None, paired_psum=True,
             load_plan="per_k", wx_bufs=2):
    """Single-core Bass program (shared by all 8 cores via SPMD).

    Inputs (per core):
      wx : (d_in, n_out + rows) bf16, packed [W.T | x_chunk.T].
    Output:
      out: (rows, n_out) bf16 = pred = x_chunk @ W.T (no bias; host adds it).
    """
    kt = d_in // 128             # 6 contraction tiles
    mblk = rows // 128           # 4 row blocks
    nchunk = n_out // nw
    wcols = n_out + rows         # 2560
    half_mb = mblk // 2          # row blocks per PSUM generation
    # paired_psum: accumulate two 512-col chunks into one 2-bank PSUM
    # tile, halving DVE copy instructions and next-generation PE WAR
    # semaphore waits (16 -> 8 per repeat).
    tw = 2 * nw if paired_psum else nw   # PSUM tile width
    cpt = tw // nw                       # matmul chunks per tile
    ncp = nchunk // cpt                  # copies per row block

    nc = bass.Bass()
    wx = nc.dram_tensor("wx", [d_in, wcols], BF16, kind="ExternalInput")
    out = nc.dram_tensor("out", [rows, n_out], BF16, kind="ExternalOutput")

    wx_r = wx.rearrange("(k p) m -> p k m", p=128)    # (128, kt, wcols)
    out_r = out.rearrange("(mb p) c -> p mb c", p=128)  # (128, mblk, n_out)

    with tile.TileContext(nc) as tc:
        with (
            tc.tile_pool(name="wxp", bufs=wx_bufs) as wxpool,
            tc.tile_pool(name="pred", bufs=2) as ppool,
            tc.tile_pool(name="psum", bufs=8 * 512 // tw,
                         space="PSUM") as pspool,
        ):
            if warm_pe:
                # The PE HAM clock-gate runs cold (1.2 GHz) until ~3.4us of
                # sustained activity.  Burn that window on dummy matmuls
                # over a zeroed tile (no load dependency, so they start at
                # t~0) so the real matmuls run at 2.4 GHz from the start.
                # Outside the repeat loop: steady-state cost is zero.
                wz = wxpool.tile([128, nw], BF16, name="wz", bufs=1)
                nc.vector.memset(wz[:], 0.0)
                wps = pspool.tile([128, tw], F32, name="ps")
                for _ in range(max(1, round(4200 / (nw + 6)))):
                    nc.tensor.matmul(wps[:, 0:nw], wz[:, 0:128], wz[:],
                                     start=True, stop=True)

            # Diagnostic builds: diag='mm' (96 matmuls only), 'ld_mm'
            # (loads + matmuls), 'mm_cp' (matmuls + copies + stores),
            # 'ld2_mm' (2 merged load DMAs + matmuls: tests per-DMA-op
            # cost), 'ldh_mm' (half the load bytes + matmuls: tests
            # byte-proportional cost).
            if pe_only:
                diag = "mm"
            do_loads = diag in (None, "ld_mm", "ld2_mm", "ldh_mm")
            do_drain = diag in (None, "mm_cp")
            if not do_loads:
                wx_pe = wxpool.tile([128, kt, wcols], BF16, name="wx_pe",
                                    bufs=1)
                nc.vector.memset(wx_pe[:], 0.0)

            for _rep in range(repeats):
                # Per-k-tile loads so the first matmul sweep can start
                # after ~1/6 of the load, overlapping the rest.  Split
                # across two DMA rings (scalar HWDGE + gpsimd SWDGE) so
                # k-tiles land ~2x faster during the fill; stores own the
                # sync ring exclusively, so cross-repeat load prefetch is
                # never FIFO-blocked behind a store.
                if not do_loads:
                    wx_sb = wx_pe
                else:
                    wx_sb = wxpool.tile([128, kt, wcols], BF16)
                    if diag == "ld2_mm":
                        h = kt // 2
                        nc.scalar.dma_start(wx_sb[:, 0:h, :],
                                            wx_r[:, 0:h, :])
                        nc.gpsimd.dma_start(wx_sb[:, h:kt, :],
                                            wx_r[:, h:kt, :])
                    elif load_plan == "scal3":
                        # 3 pair-tile DMAs, all on the scalar HWDGE ring:
                        # zero SWDGE ops (SWDGE fixed cost ~2us/op is the
                        # main measured per-op interference).  Pairs land
                        # at 3.6/7.2/10.7us -- just ahead of the k-sweep
                        # consumption schedule.
                        for j in range(0, kt, 2):
                            nc.scalar.dma_start(wx_sb[:, j:j + 2, :],
                                                wx_r[:, j:j + 2, :])
                    elif load_plan == "hwdge3":
                        # Loads on the two HWDGE rings (scalar + sync),
                        # stores displaced to gpsimd SWDGE.
                        nc.scalar.dma_start(wx_sb[:, 0:2, :],
                                            wx_r[:, 0:2, :])
                        nc.sync.dma_start(wx_sb[:, 2:4, :],
                                          wx_r[:, 2:4, :])
                        nc.scalar.dma_start(wx_sb[:, 4:kt, :],
                                            wx_r[:, 4:kt, :])
                    else:
                        lkt = kt // 2 if diag == "ldh_mm" else kt
                        for k in range(lkt):
                            eng = (nc.scalar
                                   if (k % 2 == 0 or not split_loads)
                                   else nc.gpsimd)
                            eng.dma_start(wx_sb[:, k, :], wx_r[:, k, :])

                pred = ppool.tile([128, mblk, n_out], BF16)
                st_eng = (nc.gpsimd if load_plan == "hwdge3"
                          else nc.sync)
                # Two generations of 8 concurrent PSUM banks; k-outer so
                # accumulation for all 8 chunks proceeds as k-tiles land.
                for half in range(2):
                    pss = [pspool.tile([128, tw], F32, name="ps")
                           for _ in range(half_mb * ncp)]
                    for k in range(kt):
                        for mi in range(half_mb):
                            mb = half * half_mb + mi
                            cs = n_out + mb * 128
                            for n in range(nchunk):
                                ps = pss[mi * ncp + n // cpt]
                                c0 = (n % cpt) * nw
                                nc.tensor.matmul(
                                    ps[:, c0:c0 + nw],
                                    wx_sb[:, k, cs:cs + 128],
                                    wx_sb[:, k, n * nw:(n + 1) * nw],
                                    start=(k == 0),
                                    stop=(k == kt - 1),
                                )
                    for mi in range(half_mb):
                        if not do_drain:
                            break
                        mb = half * half_mb + mi
                        for p in range(ncp):
                            # dual_copy alternates copies onto the scalar
                            # engine, but that queue also issues the even
                            # k-tile loads, so copies would FIFO-block the
                            # next repeat's load prefetch: off by default.
                            ps = pss[mi * ncp + p]
                            dst = pred[:, mb, p * tw:(p + 1) * tw]
                            if dual_copy and p % 2 == 1:
                                nc.scalar.copy(dst, ps[:])
                            else:
                                nc.vector.tensor_copy(dst, ps[:])
                            if fine_stores and ((p + 1) * tw) % 1024 == 0:
                                # Store each 1024-col slice of the row
                                # block as soon as it is copied (earlier
                                # drain, shorter tail than per-mb stores).
                                cs = (p + 1) * tw - 1024
                                st_eng.dma_start(
                                    out_r[:, mb, cs:cs + 1024],
                                    pred[:, mb, cs:cs + 1024])
                        if not fine_stores:
                            # Band store for this 128-row block.
                            st_eng.dma_start(out_r[:, mb, :],
                                             pred[:, mb, :])

    _split_multi_waits(nc)
    return nc


_CACHE = {}


def _get_nc():
    if "nc" not in _CACHE:
        _CACHE["nc"] = build_nc()
    return _CACHE["nc"]


def make_in_maps(x, W, b):
    """Host-side sharding: per-core packed [W.T | x_chunk.T] bf16."""
    x = np.asarray(x)
    W = np.asarray(W)
    WT = np.ascontiguousarray(W.T).astype(NP_BF16)    # (768, 2048)
    in_maps = []
    for c in range(N_CORES):
        bi, cc = divmod(c, CHUNKS)
        xs = x[bi, cc * ROWS:(cc + 1) * ROWS, :]
        wxc = np.empty((D_IN, N_OUT + ROWS), NP_BF16)
        wxc[:, :N_OUT] = WT
        wxc[:, N_OUT:] = np.ascontiguousarray(xs.T).astype(NP_BF16)
        in_maps.append({"wx": wxc})
    return in_maps


def unshard(results, b):
    """Scatter each core's dense band into the zero-filled full output.

    Row t's band occupies flat columns [32*t, 32*t+2048) mod 65536 of
    out[bi, t]; rows 0..1984 never wrap, so they're one strided
    (diagonal-view) assignment per batch.  Bias is added here in fp32.
    """
    b = np.asarray(b, dtype=np.float32)
    row_f = S * SPAN_DIM                       # 65536 floats per row
    nowrap = (row_f - N_OUT) // SPAN_DIM + 1   # 1985 non-wrapping rows
    out = np.zeros((B, S, S, SPAN_DIM), np.float32)
    for bi in range(B):
        band = np.concatenate(
            [np.asarray(results[bi * CHUNKS + cc]["out"]).astype(np.float32)
             for cc in range(CHUNKS)], axis=0)          # (2048, 2048)
        band += b[None, :]
        flat = out[bi].reshape(S * row_f)
        dv = np.lib.stride_tricks.as_strided(
            flat, shape=(nowrap, N_OUT),
            strides=((row_f + SPAN_DIM) * 4, 4))
        dv[:] = band[:nowrap]
        for t in range(nowrap, S):
            c0 = SPAN_DIM * t
            n1 = row_f - c0
            row = flat[t * row_f:(t + 1) * row_f]
            row[c0:] = band[t, :n1]
            row[:N_OUT - n1] = band[t, n1:]
    return out


def _run(nc, in_maps):
    return run_bass_kernel_spmd(nc, in_maps, list(range(N_CORES))).results


def kernel(x, W, b):
    x = np.asarray(x)
    W = np.asarray(W)
    b = np.asarray(b)
    nc = _get_nc()
    res = _run(nc, make_in_maps(x, W, b))
    return unshard(res, b)


# revision 40
# speedup vs baseline: 1.0246x; 1.0166x over previous
"""Trainium2 Bass kernel for nn_ExpandOperator (banded scatter of a linear projection).

Reference semantics:
    pred = x @ W.T + b                      # (B, S, 2048)
    pred = pred.reshape(B, S, 64, 32)
    out[b, t, (t+s) % S, d] = pred[b, t, s, d]   # rest of out is zeros
    out shape: (B, S, S, 32) fp32  == 1 GiB

Sharding: 8 cores = (batch b in {0,1}) x (512-row seq chunk cc in {0..3}).

Key structure: the 1 GiB output is 96.9% structural zeros — only the
(B, S, 2048)-float band carries data, and every band value is just
pred[b, t, :].  So the device computes ONLY the dense projection
pred = x @ W.T for its 512 rows (bf16 in/out; tolerance is 2e-2, bf16
error here is ~1e-3) and returns it as a compact (512, 2048) tile.
The host unshards by scattering the band into an np.zeros output —
row t's band occupies flat columns [32*t, 32*t+2048) mod 65536 of
out[b, t], which for the 1985 non-wrapping rows is a single strided
(diagonal) view assignment; the 63 wrapping rows are split copies.
The bias add (exact fp32) also folds into the host scatter:
out band row = pred_row + b.

Device per core: load [W.T | x.T] packed bf16 (6 k-tiles of 128 rows,
one DMA each so matmuls start after the first ~1.8us), 96 bf16 matmuls
(128x128x512, k-outer over 8 concurrent PSUM banks so accumulation
overlaps the remaining loads), DVE PSUM->SBUF copies (fp32->bf16), and
4 per-row-block band stores.  ~6 MB of HBM traffic and ~20.5us of PE
time per core, vs 134 MB of DMA in the write-the-zeros formulation.

The walrus build only leaves room for ONE sync-wait per compute
instruction; _split_multi_waits() hoists extra waits into same-queue
NOPs (same-queue waits execute in order, so this is semantics-neutral).
"""

import numpy as np

import bass_rust
import concourse.bass as bass
import concourse.mybir as mybir
import concourse.tile as tile
from concourse.bass_utils import run_bass_kernel_spmd

F32 = mybir.dt.float32
BF16 = mybir.dt.bfloat16
NP_BF16 = mybir.dt.np(mybir.dt.bfloat16)


def _split_multi_waits(nc):
    """Walrus in this toolchain only leaves ONE sync-wait slot per
    instruction.  Tile's tail drain waits on every semaphore lane it used,
    which fails codegen.  Hoist all-but-one wait of any multi-wait
    instruction into single-wait NOPs on the same engine queue immediately
    before it - semantically identical (same-queue waits execute in order).
    """
    eng_by_type = {
        mybir.EngineType.SP: nc.sync,
        mybir.EngineType.PE: nc.tensor,
        mybir.EngineType.Activation: nc.scalar,
        mybir.EngineType.Pool: nc.gpsimd,
        mybir.EngineType.DVE: nc.vector,
    }
    tail_bb = nc.cur_bb.bb
    for f in nc.m.functions:
        for bb in f.blocks:
            il = bb.instructions
            i = 0
            while i < len(il):
                ins = il[i]
                si = getattr(ins, "sync_info", None)
                if si is not None and len(si.on_wait) > 1:
                    waits = list(si.on_wait)
                    for w in waits[:-1]:
                        nop = eng_by_type[ins.engine].nop(nofuse=True).ins
                        tail_bb.instructions.remove(nop)
                        nop.sync_info = bass_rust.SyncInfo(
                            on_wait=[w], on_update=[])
                        il.insert(i, nop)
                        i += 1
                    ins.sync_info = bass_rust.SyncInfo(
                        on_wait=[waits[-1]], on_update=list(si.on_update))
                i += 1


# Problem shapes (hardcoded per contract).
B = 2
S = 2048
D_IN = 768
MAX_SPAN = 64
SPAN_DIM = 32
N_OUT = MAX_SPAN * SPAN_DIM  # 2048
N_CORES = 8
CHUNKS = 4                   # seq chunks per batch (B * CHUNKS == N_CORES)
ROWS = S // CHUNKS           # 512 rows per core


def build_nc(rows=ROWS, d_in=D_IN, n_out=N_OUT, repeats=1, nw=512,
             split_loads=True, fine_stores=True, dual_copy=False,
             warm_pe=True, pe_only=False, diag=None, paired_psum=True,
             load_plan="per_k", wx_bufs=2, half_stores=False):
    """Single-core Bass program (shared by all 8 cores via SPMD).

    Inputs (per core):
      wx : (d_in, n_out + rows) bf16, packed [W.T | x_chunk.T].
    Output:
      out: (rows, n_out) bf16 = pred = x_chunk @ W.T (no bias; host adds it).
    """
    kt = d_in // 128             # 6 contraction tiles
    mblk = rows // 128           # 4 row blocks
    nchunk = n_out // nw
    wcols = n_out + rows         # 2560
    half_mb = mblk // 2          # row blocks per PSUM generation
    # paired_psum: accumulate two 512-col chunks into one 2-bank PSUM
    # tile, halving DVE copy instructions and next-generation PE WAR
    # semaphore waits (16 -> 8 per repeat).
    tw = 2 * nw if paired_psum else nw   # PSUM tile width
    cpt = tw // nw                       # matmul chunks per tile
    ncp = nchunk // cpt                  # copies per row block

    nc = bass.Bass()
    wx = nc.dram_tensor("wx", [d_in, wcols], BF16, kind="ExternalInput")
    out = nc.dram_tensor("out", [rows, n_out], BF16, kind="ExternalOutput")

    wx_r = wx.rearrange("(k p) m -> p k m", p=128)    # (128, kt, wcols)
    out_r = out.rearrange("(mb p) c -> p mb c", p=128)  # (128, mblk, n_out)

    with tile.TileContext(nc) as tc:
        with (
            tc.tile_pool(name="wxp", bufs=wx_bufs) as wxpool,
            tc.tile_pool(name="pred", bufs=2) as ppool,
            tc.tile_pool(name="psum", bufs=8 * 512 // tw,
                         space="PSUM") as pspool,
        ):
            if warm_pe:
                # The PE HAM clock-gate runs cold (1.2 GHz) until ~3.4us of
                # sustained activity.  Burn that window on dummy matmuls
                # over a zeroed tile (no load dependency, so they start at
                # t~0) so the real matmuls run at 2.4 GHz from the start.
                # Outside the repeat loop: steady-state cost is zero.
                wz = wxpool.tile([128, nw], BF16, name="wz", bufs=1)
                nc.vector.memset(wz[:], 0.0)
                wps = pspool.tile([128, tw], F32, name="ps")
                for _ in range(max(1, round(4200 / (nw + 6)))):
                    nc.tensor.matmul(wps[:, 0:nw], wz[:, 0:128], wz[:],
                                     start=True, stop=True)

            # Diagnostic builds: diag='mm' (96 matmuls only), 'ld_mm'
            # (loads + matmuls), 'mm_cp' (matmuls + copies + stores),
            # 'ld2_mm' (2 merged load DMAs + matmuls: tests per-DMA-op
            # cost), 'ldh_mm' (half the load bytes + matmuls: tests
            # byte-proportional cost).
            if pe_only:
                diag = "mm"
            do_loads = diag in (None, "ld_mm", "ld2_mm", "ldh_mm")
            do_drain = diag in (None, "mm_cp")
            if not do_loads:
                wx_pe = wxpool.tile([128, kt, wcols], BF16, name="wx_pe",
                                    bufs=1)
                nc.vector.memset(wx_pe[:], 0.0)

            for _rep in range(repeats):
                # Per-k-tile loads so the first matmul sweep can start
                # after ~1/6 of the load, overlapping the rest.  Split
                # across two DMA rings (scalar HWDGE + gpsimd SWDGE) so
                # k-tiles land ~2x faster during the fill; stores own the
                # sync ring exclusively, so cross-repeat load prefetch is
                # never FIFO-blocked behind a store.
                if not do_loads:
                    wx_sb = wx_pe
                else:
                    wx_sb = wxpool.tile([128, kt, wcols], BF16)
                    if diag == "ld2_mm":
                        h = kt // 2
                        nc.scalar.dma_start(wx_sb[:, 0:h, :],
                                            wx_r[:, 0:h, :])
                        nc.gpsimd.dma_start(wx_sb[:, h:kt, :],
                                            wx_r[:, h:kt, :])
                    elif load_plan == "tiny":
                        # Diagnostic: full pipeline but loads move only
                        # one k-tile of bytes (matmuls read stale data
                        # for k>0 -- timing-only build).
                        nc.scalar.dma_start(wx_sb[:, 0:1, :],
                                            wx_r[:, 0:1, :])
                    elif load_plan == "two":
                        # Op-minimal: one 3-tile DMA per ring.
                        h = kt // 2
                        nc.scalar.dma_start(wx_sb[:, 0:h, :],
                                            wx_r[:, 0:h, :])
                        nc.gpsimd.dma_start(wx_sb[:, h:kt, :],
                                            wx_r[:, h:kt, :])
                    elif load_plan == "scal3":
                        # 3 pair-tile DMAs, all on the scalar HWDGE ring:
                        # zero SWDGE ops (SWDGE fixed cost ~2us/op is the
                        # main measured per-op interference).  Pairs land
                        # at 3.6/7.2/10.7us -- just ahead of the k-sweep
                        # consumption schedule.
                        for j in range(0, kt, 2):
                            nc.scalar.dma_start(wx_sb[:, j:j + 2, :],
                                                wx_r[:, j:j + 2, :])
                    elif load_plan == "hwdge3":
                        # Loads on the two HWDGE rings (scalar + sync),
                        # stores displaced to gpsimd SWDGE.
                        nc.scalar.dma_start(wx_sb[:, 0:2, :],
                                            wx_r[:, 0:2, :])
                        nc.sync.dma_start(wx_sb[:, 2:4, :],
                                          wx_r[:, 2:4, :])
                        nc.scalar.dma_start(wx_sb[:, 4:kt, :],
                                            wx_r[:, 4:kt, :])
                    else:
                        lkt = kt // 2 if diag == "ldh_mm" else kt
                        for k in range(lkt):
                            eng = (nc.scalar
                                   if (k % 2 == 0 or not split_loads)
                                   else nc.gpsimd)
                            eng.dma_start(wx_sb[:, k, :], wx_r[:, k, :])

                pred = ppool.tile([128, mblk, n_out], BF16)
                st_eng = (nc.gpsimd if load_plan == "hwdge3"
                          else nc.sync)
                # Two generations of 8 concurrent PSUM banks; k-outer so
                # accumulation for all 8 chunks proceeds as k-tiles land.
                for half in range(2):
                    pss = [pspool.tile([128, tw], F32, name="ps")
                           for _ in range(half_mb * ncp)]
                    for k in range(kt):
                        for mi in range(half_mb):
                            mb = half * half_mb + mi
                            cs = n_out + mb * 128
                            for n in range(nchunk):
                                ps = pss[mi * ncp + n // cpt]
                                c0 = (n % cpt) * nw
                                nc.tensor.matmul(
                                    ps[:, c0:c0 + nw],
                                    wx_sb[:, k, cs:cs + 128],
                                    wx_sb[:, k, n * nw:(n + 1) * nw],
                                    start=(k == 0),
                                    stop=(k == kt - 1),
                                )
                    for mi in range(half_mb):
                        if not do_drain:
                            break
                        mb = half * half_mb + mi
                        for p in range(ncp):
                            # dual_copy alternates copies onto the scalar
                            # engine, but that queue also issues the even
                            # k-tile loads, so copies would FIFO-block the
                            # next repeat's load prefetch: off by default.
                            ps = pss[mi * ncp + p]
                            dst = pred[:, mb, p * tw:(p + 1) * tw]
                            if dual_copy and p % 2 == 1:
                                nc.scalar.copy(dst, ps[:])
                            else:
                                nc.vector.tensor_copy(dst, ps[:])
                            if fine_stores and ((p + 1) * tw) % 1024 == 0:
                                # Store each 1024-col slice of the row
                                # block as soon as it is copied (earlier
                                # drain, shorter tail than per-mb stores).
                                cs = (p + 1) * tw - 1024
                                st_eng.dma_start(
                                    out_r[:, mb, cs:cs + 1024],
                                    pred[:, mb, cs:cs + 1024])
                        if not fine_stores and not half_stores:
                            # Band store for this 128-row block.
                            st_eng.dma_start(out_r[:, mb, :],
                                             pred[:, mb, :])
                    if half_stores and do_drain:
                        # Op-minimal: one 1MB store per generation
                        # covering both of its row blocks.
                        m0 = half * half_mb
                        st_eng.dma_start(
                            out_r[:, m0:m0 + half_mb, :],
                            pred[:, m0:m0 + half_mb, :])

    _split_multi_waits(nc)
    return nc


_CACHE = {}


def _get_nc():
    if "nc" not in _CACHE:
        _CACHE["nc"] = build_nc()
    return _CACHE["nc"]


def make_in_maps(x, W, b):
    """Host-side sharding: per-core packed [W.T | x_chunk.T] bf16."""
    x = np.asarray(x)
    W = np.asarray(W)
    WT = np.ascontiguousarray(W.T).astype(NP_BF16)    # (768, 2048)
    in_maps = []
    for c in range(N_CORES):
        bi, cc = divmod(c, CHUNKS)
        xs = x[bi, cc * ROWS:(cc + 1) * ROWS, :]
        wxc = np.empty((D_IN, N_OUT + ROWS), NP_BF16)
        wxc[:, :N_OUT] = WT
        wxc[:, N_OUT:] = np.ascontiguousarray(xs.T).astype(NP_BF16)
        in_maps.append({"wx": wxc})
    return in_maps


def unshard(results, b):
    """Scatter each core's dense band into the zero-filled full output.

    Row t's band occupies flat columns [32*t, 32*t+2048) mod 65536 of
    out[bi, t]; rows 0..1984 never wrap, so they're one strided
    (diagonal-view) assignment per batch.  Bias is added here in fp32.
    """
    b = np.asarray(b, dtype=np.float32)
    row_f = S * SPAN_DIM                       # 65536 floats per row
    nowrap = (row_f - N_OUT) // SPAN_DIM + 1   # 1985 non-wrapping rows
    out = np.zeros((B, S, S, SPAN_DIM), np.float32)
    for bi in range(B):
        band = np.concatenate(
            [np.asarray(results[bi * CHUNKS + cc]["out"]).astype(np.float32)
             for cc in range(CHUNKS)], axis=0)          # (2048, 2048)
        band += b[None, :]
        flat = out[bi].reshape(S * row_f)
        dv = np.lib.stride_tricks.as_strided(
            flat, shape=(nowrap, N_OUT),
            strides=((row_f + SPAN_DIM) * 4, 4))
        dv[:] = band[:nowrap]
        for t in range(nowrap, S):
            c0 = SPAN_DIM * t
            n1 = row_f - c0
            row = flat[t * row_f:(t + 1) * row_f]
            row[c0:] = band[t, :n1]
            row[:N_OUT - n1] = band[t, n1:]
    return out


def _run(nc, in_maps):
    return run_bass_kernel_spmd(nc, in_maps, list(range(N_CORES))).results


def kernel(x, W, b):
    x = np.asarray(x)
    W = np.asarray(W)
    b = np.asarray(b)
    nc = _get_nc()
    res = _run(nc, make_in_maps(x, W, b))
    return unshard(res, b)
